# revision 24
# baseline (speedup 1.0000x reference)
"""Additive (Bahdanau) attention kernel for 8 Trainium2 NeuronCores.

Problem shapes (hardcoded): B=64, T=2048, Q_DIM=K_DIM=H_DIM=1024.
  q_proj = query @ Wq + bq                      (B, H)
  k_proj = keys @ Wk + bk                       (B, T, H)
  energy = tanh(q_proj[:, None, :] + k_proj)    (B, T, H)
  scores = energy @ Wv + bv                     (B, T)
  attn   = softmax(scores, axis=1)              (B, T)
  context= attn @ values                        (B, V)
returns (context, attn).

Strategy: data-parallel over batch, 8 batches per core.
 - Host: fold q_proj+bq+bk into a per-(batch,h) bias (fp32, exact); bv
   drops out of the softmax entirely. Cast keys/values/Wk to bf16 and
   pre-transpose keys to [K, T] per batch so the device only does clean
   contiguous DMAs (128KB chunks for fine-grained prefetch).
 - Device per batch: k_proj via PE matmuls with Wk stationary
   ([h partitions, t free] orientation, fp32 PSUM accum over K chunks);
   tanh+bias fused in one ScalarE activation per tile (bias is
   per-partition = per-h).
 - Scores: ScalarE scales energy by Wv (per-partition scale), DVE sums
   the 8 h-chunks (fp32 chain, last add emits fp16), and a single
   ones-vector fp16 matmul does the 128-partition reduction at full
   streaming rate. The reduce work is emitted TWO t-tile stages late so
   the in-order PE never waits on the ACT/DVE chain.
 - Softmax online, no max subtraction (|scores| <= sum|Wv| < 32, exp is
   safe in fp32): exp per t-tile straight out of PSUM with accum_out
   partial sums. The 1/sum normalization folds into the final scale of
   attn and context.
 - Context: DVE multiplies values (native [t, v] layout) by the
   unnormalized attn weight (per-partition scalar, via a tiny DRAM
   transpose roundtrip), 4 t-chunks per reduce stage, fp32 chain with a
   fp16 tail, ones-fp16 matmul reduces partitions one batch later.
"""

import os
import sys
from contextlib import ExitStack

for _p in ("/opt/trn_rl_repo", "/root/.axon_site/_ro/trn_rl_repo"):
    if os.path.isdir(_p) and _p not in sys.path:
        sys.path.append(_p)

import numpy as np
import ml_dtypes

BF16 = ml_dtypes.bfloat16

N_CORES = 8
B, T, H, KD = 64, 2048, 1024, 1024
NB = B // N_CORES          # batches per core
KO = KD // 128             # contraction chunks (k on partitions)
HT = H // 128              # h tiles (output partitions)
NT = 512                   # matmul moving free dim
TT = T // NT               # t tiles per batch
TC = T // 128              # t chunks (contraction for context)
TCS = TC // TT             # t chunks per t tile (4)

_CACHE = {}


def _build():
    """Build + bacc-compile the SPMD Bass program (once per process)."""
    import concourse.bacc as bacc
    import concourse.tile as tile
    from concourse import mybir

    f32 = mybir.dt.float32
    f16 = mybir.dt.float16
    bf16 = mybir.dt.bfloat16
    Tanh = mybir.ActivationFunctionType.Tanh
    Exp = mybir.ActivationFunctionType.Exp
    Copy = mybir.ActivationFunctionType.Copy
    add = mybir.AluOpType.add

    nc = bacc.Bacc("TRN2", target_bir_lowering=False, debug=False,
                   num_devices=N_CORES)

    kT_d = nc.dram_tensor("kT", [NB, KD, T], bf16, kind="ExternalInput").ap()
    val_d = nc.dram_tensor("vals", [NB, T, H], bf16, kind="ExternalInput").ap()
    wk_d = nc.dram_tensor("wk", [KD, H], bf16, kind="ExternalInput").ap()
    qbT_d = nc.dram_tensor("qbT", [128, NB * HT], f32, kind="ExternalInput").ap()
    wvT_d = nc.dram_tensor("wvT", [128, HT], f32, kind="ExternalInput").ap()
    ctx_d = nc.dram_tensor("ctx", [NB, H], f32, kind="ExternalOutput").ap()
    attn_d = nc.dram_tensor("attn", [NB, T], f32, kind="ExternalOutput").ap()
    ef_d = nc.dram_tensor("ef", [NB, T], f32).ap()      # internal scratch

    kT3 = kT_d.rearrange("b (ko p) t -> b ko p t", p=128)
    val3 = val_d.rearrange("b (tc p) v -> b p tc v", p=128)
    wk3 = wk_d.rearrange("(ko p) h -> ko p h", p=128)

    with tile.TileContext(nc) as tc, ExitStack() as ctx:
        const = ctx.enter_context(tc.tile_pool(name="const", bufs=1))
        kt_pool = ctx.enter_context(tc.tile_pool(name="kt", bufs=20))
        val_pool = ctx.enter_context(tc.tile_pool(name="val", bufs=2))
        en_pool = ctx.enter_context(tc.tile_pool(name="en", bufs=3))
        enw_pool = ctx.enter_context(tc.tile_pool(name="enw", bufs=4))
        acc_pool = ctx.enter_context(tc.tile_pool(name="acc", bufs=3))
        a16_pool = ctx.enter_context(tc.tile_pool(name="a16", bufs=3))
        prod_pool = ctx.enter_context(tc.tile_pool(name="prod", bufs=3))
        accv_pool = ctx.enter_context(tc.tile_pool(name="accv", bufs=2))
        av16_pool = ctx.enter_context(tc.tile_pool(name="av16", bufs=2))
        at_pool = ctx.enter_context(tc.tile_pool(name="at", bufs=3))
        sm_pool = ctx.enter_context(tc.tile_pool(name="sm", bufs=3))
        ef_pool = ctx.enter_context(tc.tile_pool(name="efp", bufs=2))
        out_pool = ctx.enter_context(tc.tile_pool(name="outp", bufs=2))
        kp_psum = ctx.enter_context(tc.tile_pool(name="kp", bufs=5, space="PSUM"))
        vp_psum = ctx.enter_context(tc.tile_pool(name="vp", bufs=2, space="PSUM"))
        tp_psum = ctx.enter_context(tc.tile_pool(name="tp", bufs=1, space="PSUM"))

        # prefetch the very first k-chunk tiles ahead of the fat constant
        # DMAs so the PE can start within a few microseconds
        kt_first = {}
        for ko in range(KO):
            kc = kt_pool.tile([128, NT], bf16, tag="kt", name=f"kt0_{ko}_0")
            nc.sync.dma_start(kc[:], kT3[0, ko][:, 0:NT])
            kt_first[(ko, 0)] = kc
        # resident constants; Wk in per-ko chunks so the first matmul only
        # waits on a 256KB DMA
        wk_sb = []
        for ko in range(KO):
            w = const.tile([128, H], bf16, tag=f"wk{ko}")
            nc.sync.dma_start(w[:], wk3[ko])
            wk_sb.append(w)
        qbT_sb = const.tile([128, NB * HT], f32, tag="qbT")
        nc.sync.dma_start(qbT_sb[:], qbT_d[:])
        wvT_sb = const.tile([128, HT], f32, tag="wvT")
        nc.sync.dma_start(wvT_sb[:], wvT_d[:])
        ones16 = const.tile([128, 1], f16, tag="ones16")
        nc.vector.memset(ones16[:], 1.0)
        one11 = const.tile([1, 1], f32, tag="one11")
        nc.vector.memset(one11[:], 1.0)

        # per-batch state
        st = {}   # b -> dict(kt{}, a16{tt}, e_f, S4, aT, rs, val, ...)

        def tt_block(b, tt):
            """k_proj + tanh + Wv-scale + h-chunk accumulation for (b, tt)."""
            s = st[b]
            tsl = slice(tt * NT, (tt + 1) * NT)
            acc = acc_pool.tile([128, NT], f16, tag="acc", name=f"acc{b}_{tt}")
            a16 = a16_pool.tile([128, NT], f16, tag="a16", name=f"a16_{b}_{tt}")
            s["a16"][tt] = a16
            enw_prev = None
            for ht in range(HT):
                kp = kp_psum.tile([128, NT], f32, tag="kp")
                for ko in range(KO):
                    nc.tensor.matmul(
                        kp[:],
                        wk_sb[ko][:, ht * 128:(ht + 1) * 128],
                        s["kt"][(ko, tt)][:],
                        start=(ko == 0), stop=(ko == KO - 1))
                en = en_pool.tile([128, NT], bf16, tag="en")
                nc.scalar.activation(
                    en[:], kp[:], Tanh,
                    bias=qbT_sb[:, b * HT + ht:b * HT + ht + 1])
                enw = enw_pool.tile([128, NT], f16, tag="enw")
                nc.vector.tensor_scalar_mul(enw[:], en[:],
                                            wvT_sb[:, ht:ht + 1])
                if ht == 0:
                    enw_prev = enw
                elif ht == 1:
                    nc.vector.tensor_tensor(acc[:], enw_prev[:], enw[:], add)
                elif ht < HT - 1:
                    nc.vector.tensor_tensor(acc[:], acc[:], enw[:], add)
                else:
                    nc.vector.tensor_tensor(a16[:], acc[:], enw[:], add)

        def ctx_chunk(b, t, prod):
            """advance the context accumulation chain by one t-chunk."""
            s = st[b]
            if t == 0:
                s["cprev"] = prod
            elif t == 1:
                nc.vector.tensor_tensor(s["caccv"][:], s["cprev"][:],
                                        prod[:], add)
            elif t < TC - 1:
                nc.vector.tensor_tensor(s["caccv"][:], s["caccv"][:],
                                        prod[:], add)
            else:
                nc.vector.tensor_tensor(s["cav16"][:], s["caccv"][:],
                                        prod[:], add)

        def reduce_stage(b, tt):
            """Partition-reduce + exp + attn scatter for (b, tt) + context
            chunk muls; emitted two t-tile stages late so the in-order PE
            never waits on the ACT/DVE chain."""
            s = st[b]
            tsl = slice(tt * NT, (tt + 1) * NT)
            a16 = s["a16"].pop(tt)
            spt = vp_psum.tile([1, NT], f32, tag="vp")
            nc.tensor.matmul(spt[:], ones16[:], a16[:], start=True, stop=True)
            nc.scalar.activation(s["e_f"][:, tsl], spt[:], Exp,
                                 accum_out=s["S4"][:, tt:tt + 1])
            if b == NB - 1:
                # tail path: PE-transpose the 4 attn chunks (no DMA latency)
                atp = tp_psum.tile([128, TCS], f32, tag="tp")
                for j in range(TCS):
                    tch = (tt * TCS + j) * 128
                    nc.tensor.matmul(atp[:, j:j + 1],
                                     s["e_f"][0:1, tch:tch + 128], one11[:],
                                     is_transpose=True, start=True, stop=True)
                nc.scalar.copy(s["aT"][:, tt * TCS:(tt + 1) * TCS], atp[:])
            else:
                nc.sync.dma_start(ef_d[b:b + 1, tsl], s["e_f"][:, tsl])
                with nc.allow_non_contiguous_dma(reason="tiny 2KB attn transpose"):
                    nc.gpsimd.dma_start(
                        s["aT"][:, tt * TCS:(tt + 1) * TCS],
                        ef_d[b, tsl].rearrange("(c p) -> p c", p=128))
            if tt == TT - 1:
                # batch fully scored: finalize the softmax scalars first so
                # the context chain below never waits on the attn output
                S = sm_pool.tile([1, 1], f32, tag="S")
                nc.vector.reduce_sum(S[:], s["S4"][:], axis=mybir.AxisListType.X)
                rs = sm_pool.tile([1, 1], f32, tag="rs")
                nc.vector.reciprocal(rs[:], S[:])
                s["rs"] = rs
            # context products for the 4 t-chunks just scattered into aT.
            # The last stage's products are deferred one more stage so the
            # in-order DVE never waits on the attn-gather DMA latency.
            if tt < TT - 1:
                emit_ctx_prods(b, tt)
            else:
                late_ctx.append(b)
                att_f = out_pool.tile([1, T], f32, tag="attf")
                nc.vector.tensor_scalar_mul(att_f[:], s["e_f"][:], s["rs"][:])
                nc.sync.dma_start(attn_d[b:b + 1, :], att_f[:])

        def emit_ctx_prods(b, tt):
            s = st[b]
            for t in range(tt * TCS, (tt + 1) * TCS):
                prod = prod_pool.tile([128, H], f16, tag="prod")
                # products on ScalarE (it has slack; DVE is lockstepped
                # behind tanh for the scores chain), chain adds stay on DVE
                nc.scalar.activation(prod[:], s["val"][:, t, :], Copy,
                                     scale=s["aT"][:, t:t + 1])
                ctx_chunk(b, t, prod)

        def flush_late_ctx():
            while late_ctx:
                emit_ctx_prods(late_ctx.pop(0), TT - 1)

        def ctx_pe(b):
            """partition-reduce + normalize + store context (PE/ACT tail)."""
            s = st.pop(b)
            cs = out_pool.tile([1, H], f32, tag="cs")
            for vh in range(2):
                cp = vp_psum.tile([1, NT], f32, tag="vp")
                nc.tensor.matmul(cp[:], ones16[:],
                                 s["cav16"][:, vh * NT:(vh + 1) * NT],
                                 start=True, stop=True)
                # fold the softmax 1/sum into the copy out of PSUM
                nc.scalar.activation(cs[:, vh * NT:(vh + 1) * NT], cp[:],
                                     Copy, scale=s["rs"][:])
            nc.sync.dma_start(ctx_d[b:b + 1, :], cs[:])

        from collections import deque
        pending = deque()   # (b, tt) blocks whose reduce stage is deferred
        late_ctx = []       # batches whose tt3 ctx products are deferred

        for b in range(NB):
            e_f = ef_pool.tile([1, T], f32, tag="ef", name=f"ef{b}")
            S4 = sm_pool.tile([1, TT], f32, tag="S4", name=f"S4_{b}")
            aT = at_pool.tile([128, TC], f32, tag="aT", name=f"aT{b}")
            caccv = accv_pool.tile([128, H], f16, tag="accv", name=f"cv{b}")
            cav16 = av16_pool.tile([128, H], f16, tag="av16", name=f"cv16_{b}")
            st[b] = {"kt": dict(kt_first) if b == 0 else {}, "a16": {},
                     "e_f": e_f, "S4": S4, "aT": aT,
                     "caccv": caccv, "cav16": cav16}

            for tt in range(TT):
                if not (b == 0 and tt == 0):
                    for ko in range(KO):
                        kc = kt_pool.tile([128, NT], bf16, tag="kt",
                                          name=f"kt{b}_{ko}_{tt}")
                        nc.sync.dma_start(kc[:],
                                          kT3[b, ko][:, tt * NT:(tt + 1) * NT])
                        st[b]["kt"][(ko, tt)] = kc
                tt_block(b, tt)
                if tt == 0:
                    val_t = val_pool.tile([128, TC, H], bf16, tag="val",
                                          name=f"val{b}")
                    st[b]["val"] = val_t
                # values arrive in 1MB chunks, each two stages before its use
                nc.sync.dma_start(
                    st[b]["val"][:, tt * TCS:(tt + 1) * TCS, :],
                    val3[b][:, tt * TCS:(tt + 1) * TCS, :])
                pending.append((b, tt))
                if len(pending) > 2:
                    reduce_stage(*pending.popleft())
                    if pending[-1][1] == 2:
                        # one stage after the previous batch's tt3 reduce:
                        # its attn gather has long landed
                        flush_late_ctx()
                # drain one stage early at the very end so the epilogue only
                # has a single serialized reduce chain left
                if b == NB - 1 and tt == TT - 1:
                    reduce_stage(*pending.popleft())
                # free this t-tile's key chunks
                for ko in range(KO):
                    del st[b]["kt"][(ko, tt)]
            if b > 0:
                ctx_pe(b - 1)

        while pending:
            reduce_stage(*pending.popleft())
        flush_late_ctx()
        ctx_pe(NB - 1)

    nc.compile()
    return nc


def _prep(query, keys, values, Wq, bq, Wk, bk, Wv, bv):
    qb = (query.astype(np.float32) @ Wq.astype(np.float32)
          + bq.astype(np.float32) + bk.astype(np.float32))       # [B, H]
    wk_bf = np.ascontiguousarray(Wk.astype(BF16))
    wvT = np.ascontiguousarray(Wv.reshape(HT, 128).T.astype(np.float32))
    in_maps = []
    for c in range(N_CORES):
        sl = slice(c * NB, (c + 1) * NB)
        kT = np.ascontiguousarray(
            keys[sl].transpose(0, 2, 1).astype(BF16))            # [NB, K, T]
        vals = np.ascontiguousarray(values[sl].astype(BF16))     # [NB, T, H]
        qbT = np.ascontiguousarray(
            qb[sl].reshape(NB, HT, 128).transpose(2, 0, 1).reshape(128, NB * HT))
        in_maps.append({"kT": kT, "vals": vals, "wk": wk_bf,
                        "qbT": qbT, "wvT": wvT})
    return in_maps


def kernel(query, keys, values, Wq, bq, Wk, bk, Wv, bv):
    from concourse.bass_utils import run_bass_kernel_spmd

    if "nc" not in _CACHE:
        _CACHE["nc"] = _build()
    nc = _CACHE["nc"]

    in_maps = _prep(query, keys, values, Wq, bq, Wk, bk, Wv, bv)
    trace = bool(int(os.environ.get("KERNEL_TRACE", "0")))
    res = run_bass_kernel_spmd(nc, in_maps, core_ids=list(range(N_CORES)),
                               trace=trace)
    _CACHE["last_exec_ns"] = res.exec_time_ns
    _CACHE["last_results"] = res

    context = np.concatenate([res.results[c]["ctx"] for c in range(N_CORES)], axis=0)
    attn = np.concatenate([res.results[c]["attn"] for c in range(N_CORES)], axis=0)
    return context.astype(np.float32), attn.astype(np.float32)


# revision 25
# speedup vs baseline: 1.2242x; 1.2242x over previous
"""Additive (Bahdanau) attention kernel for 8 Trainium2 NeuronCores.

Problem shapes (hardcoded): B=64, T=2048, Q_DIM=K_DIM=H_DIM=1024.
  q_proj = query @ Wq + bq                      (B, H)
  k_proj = keys @ Wk + bk                       (B, T, H)
  energy = tanh(q_proj[:, None, :] + k_proj)    (B, T, H)
  scores = energy @ Wv + bv                     (B, T)
  attn   = softmax(scores, axis=1)              (B, T)
  context= attn @ values                        (B, V)
returns (context, attn).

Strategy: data-parallel over batch, 8 batches per core.
 - Host: fold q_proj+bq+bk into a per-(batch,h) bias (fp32, exact); bv
   drops out of the softmax entirely. Cast keys/values/Wk to bf16 and
   pre-transpose keys to [K, T] per batch so the device only does clean
   contiguous DMAs (128KB chunks for fine-grained prefetch).
 - Device per batch: k_proj via PE matmuls with Wk stationary
   ([h partitions, t free] orientation, fp32 PSUM accum over K chunks);
   tanh+bias fused in one ScalarE activation per tile (bias is
   per-partition = per-h).
 - Scores: ScalarE scales energy by Wv (per-partition scale), DVE sums
   the 8 h-chunks (fp32 chain, last add emits fp16), and a single
   ones-vector fp16 matmul does the 128-partition reduction at full
   streaming rate. The reduce work is emitted TWO t-tile stages late so
   the in-order PE never waits on the ACT/DVE chain.
 - Softmax online, no max subtraction (|scores| <= sum|Wv| < 32, exp is
   safe in fp32): exp per t-tile straight out of PSUM with accum_out
   partial sums. The 1/sum normalization folds into the final scale of
   attn and context.
 - Context: DVE multiplies values (native [t, v] layout) by the
   unnormalized attn weight (per-partition scalar, via a tiny DRAM
   transpose roundtrip), 4 t-chunks per reduce stage, fp32 chain with a
   fp16 tail, ones-fp16 matmul reduces partitions one batch later.
"""

import os
import sys
from contextlib import ExitStack

for _p in ("/opt/trn_rl_repo", "/root/.axon_site/_ro/trn_rl_repo"):
    if os.path.isdir(_p) and _p not in sys.path:
        sys.path.append(_p)

import numpy as np
import ml_dtypes

BF16 = ml_dtypes.bfloat16

N_CORES = 8
B, T, H, KD = 64, 2048, 1024, 1024
NB = B // N_CORES          # batches per core
KO = KD // 128             # contraction chunks (k on partitions)
HT = H // 128              # h tiles (output partitions)
NT = 512                   # matmul moving free dim
TT = T // NT               # t tiles per batch
TC = T // 128              # t chunks (contraction for context)
TCS = TC // TT             # t chunks per t tile (4)

_CACHE = {}


def _build():
    """Build + bacc-compile the SPMD Bass program (once per process)."""
    import concourse.bacc as bacc
    import concourse.tile as tile
    from concourse import mybir

    f32 = mybir.dt.float32
    f16 = mybir.dt.float16
    bf16 = mybir.dt.bfloat16
    Tanh = mybir.ActivationFunctionType.Tanh
    Exp = mybir.ActivationFunctionType.Exp
    Copy = mybir.ActivationFunctionType.Copy
    add = mybir.AluOpType.add

    nc = bacc.Bacc("TRN2", target_bir_lowering=False, debug=False,
                   num_devices=N_CORES)

    kT_d = nc.dram_tensor("kT", [NB, KD, T], bf16, kind="ExternalInput").ap()
    val_d = nc.dram_tensor("vals", [NB, T, H], bf16, kind="ExternalInput").ap()
    wk_d = nc.dram_tensor("wk", [KD, H], bf16, kind="ExternalInput").ap()
    qbT_d = nc.dram_tensor("qbT", [128, NB * HT], f32, kind="ExternalInput").ap()
    wvT_d = nc.dram_tensor("wvT", [128, HT], f32, kind="ExternalInput").ap()
    ctx_d = nc.dram_tensor("ctx", [NB, H], f32, kind="ExternalOutput").ap()
    attn_d = nc.dram_tensor("attn", [NB, T], f32, kind="ExternalOutput").ap()
    ef_d = nc.dram_tensor("ef", [NB, T], f32).ap()      # internal scratch

    kT3 = kT_d.rearrange("b (ko p) t -> b ko p t", p=128)
    val3 = val_d.rearrange("b (tc p) v -> b p tc v", p=128)
    wk3 = wk_d.rearrange("(ko p) h -> ko p h", p=128)

    with tile.TileContext(nc) as tc, ExitStack() as ctx:
        const = ctx.enter_context(tc.tile_pool(name="const", bufs=1))
        kt_pool = ctx.enter_context(tc.tile_pool(name="kt", bufs=20))
        val_pool = ctx.enter_context(tc.tile_pool(name="val", bufs=2))
        en_pool = ctx.enter_context(tc.tile_pool(name="en", bufs=3))
        enw_pool = ctx.enter_context(tc.tile_pool(name="enw", bufs=4))
        acc_pool = ctx.enter_context(tc.tile_pool(name="acc", bufs=3))
        a16_pool = ctx.enter_context(tc.tile_pool(name="a16", bufs=3))
        prod_pool = ctx.enter_context(tc.tile_pool(name="prod", bufs=3))
        accv_pool = ctx.enter_context(tc.tile_pool(name="accv", bufs=2))
        av16_pool = ctx.enter_context(tc.tile_pool(name="av16", bufs=2))
        at_pool = ctx.enter_context(tc.tile_pool(name="at", bufs=3))
        sm_pool = ctx.enter_context(tc.tile_pool(name="sm", bufs=3))
        ef_pool = ctx.enter_context(tc.tile_pool(name="efp", bufs=2))
        out_pool = ctx.enter_context(tc.tile_pool(name="outp", bufs=2))
        kp_psum = ctx.enter_context(tc.tile_pool(name="kp", bufs=5, space="PSUM"))
        vp_psum = ctx.enter_context(tc.tile_pool(name="vp", bufs=2, space="PSUM"))
        tp_psum = ctx.enter_context(tc.tile_pool(name="tp", bufs=1, space="PSUM"))

        # prefetch the very first k-chunk tiles ahead of the fat constant
        # DMAs so the PE can start within a few microseconds
        kt_first = {}
        for ko in range(KO):
            kc = kt_pool.tile([128, NT], bf16, tag="kt", name=f"kt0_{ko}_0")
            nc.sync.dma_start(kc[:], kT3[0, ko][:, 0:NT])
            kt_first[(ko, 0)] = kc
        # resident constants; Wk in per-ko chunks so the first matmul only
        # waits on a 256KB DMA
        wk_sb = []
        for ko in range(KO):
            w = const.tile([128, H], bf16, tag=f"wk{ko}")
            nc.sync.dma_start(w[:], wk3[ko])
            wk_sb.append(w)
        qbT_sb = const.tile([128, NB * HT], f32, tag="qbT")
        nc.sync.dma_start(qbT_sb[:], qbT_d[:])
        wvT_sb = const.tile([128, HT], f32, tag="wvT")
        nc.sync.dma_start(wvT_sb[:], wvT_d[:])
        ones16 = const.tile([128, 1], f16, tag="ones16")
        nc.vector.memset(ones16[:], 1.0)
        one11 = const.tile([1, 1], f32, tag="one11")
        nc.vector.memset(one11[:], 1.0)

        # per-batch state
        st = {}   # b -> dict(kt{}, a16{tt}, e_f, S4, aT, rs, val, ...)

        def tt_block(b, tt):
            """k_proj + tanh + Wv-scale + h-chunk accumulation for (b, tt)."""
            s = st[b]
            tsl = slice(tt * NT, (tt + 1) * NT)
            acc = acc_pool.tile([128, NT], f16, tag="acc", name=f"acc{b}_{tt}")
            a16 = a16_pool.tile([128, NT], f16, tag="a16", name=f"a16_{b}_{tt}")
            s["a16"][tt] = a16
            enw_prev = None
            for ht in range(HT):
                kp = kp_psum.tile([128, NT], f32, tag="kp")
                for ko in range(KO):
                    nc.tensor.matmul(
                        kp[:],
                        wk_sb[ko][:, ht * 128:(ht + 1) * 128],
                        s["kt"][(ko, tt)][:],
                        start=(ko == 0), stop=(ko == KO - 1))
                en = en_pool.tile([128, NT], bf16, tag="en")
                nc.scalar.activation(
                    en[:], kp[:], Tanh,
                    bias=qbT_sb[:, b * HT + ht:b * HT + ht + 1])
                enw = enw_pool.tile([128, NT], f16, tag="enw")
                nc.vector.tensor_scalar_mul(enw[:], en[:],
                                            wvT_sb[:, ht:ht + 1])
                if ht == 0:
                    enw_prev = enw
                elif ht == 1:
                    nc.vector.tensor_tensor(acc[:], enw_prev[:], enw[:], add)
                elif ht < HT - 1:
                    nc.vector.tensor_tensor(acc[:], acc[:], enw[:], add)
                else:
                    nc.vector.tensor_tensor(a16[:], acc[:], enw[:], add)

        def ctx_chunk(b, t, prod):
            """advance the context accumulation chain by one t-chunk."""
            s = st[b]
            if t == 0:
                s["cprev"] = prod
            elif t == 1:
                nc.vector.tensor_tensor(s["caccv"][:], s["cprev"][:],
                                        prod[:], add)
            elif t < TC - 1:
                nc.vector.tensor_tensor(s["caccv"][:], s["caccv"][:],
                                        prod[:], add)
            else:
                nc.vector.tensor_tensor(s["cav16"][:], s["caccv"][:],
                                        prod[:], add)

        def reduce_stage(b, tt):
            """Partition-reduce + exp + attn scatter for (b, tt) + context
            chunk muls; emitted two t-tile stages late so the in-order PE
            never waits on the ACT/DVE chain."""
            s = st[b]
            tsl = slice(tt * NT, (tt + 1) * NT)
            a16 = s["a16"].pop(tt)
            spt = vp_psum.tile([1, NT], f32, tag="vp")
            nc.tensor.matmul(spt[:], ones16[:], a16[:], start=True, stop=True)
            nc.scalar.activation(s["e_f"][:, tsl], spt[:], Exp,
                                 accum_out=s["S4"][:, tt:tt + 1])
            if b == NB - 1:
                # tail path: PE-transpose the 4 attn chunks (no DMA latency)
                atp = tp_psum.tile([128, TCS], f32, tag="tp")
                for j in range(TCS):
                    tch = (tt * TCS + j) * 128
                    nc.tensor.matmul(atp[:, j:j + 1],
                                     s["e_f"][0:1, tch:tch + 128], one11[:],
                                     is_transpose=True, start=True, stop=True)
                nc.scalar.copy(s["aT"][:, tt * TCS:(tt + 1) * TCS], atp[:])
            else:
                nc.sync.dma_start(ef_d[b:b + 1, tsl], s["e_f"][:, tsl])
                with nc.allow_non_contiguous_dma(reason="tiny 2KB attn transpose"):
                    nc.gpsimd.dma_start(
                        s["aT"][:, tt * TCS:(tt + 1) * TCS],
                        ef_d[b, tsl].rearrange("(c p) -> p c", p=128))
            if tt == TT - 1:
                # batch fully scored: finalize the softmax scalars first so
                # the context chain below never waits on the attn output
                S = sm_pool.tile([1, 1], f32, tag="S")
                nc.vector.reduce_sum(S[:], s["S4"][:], axis=mybir.AxisListType.X)
                rs = sm_pool.tile([1, 1], f32, tag="rs")
                nc.vector.reciprocal(rs[:], S[:])
                s["rs"] = rs
            # context products for the 4 t-chunks just scattered into aT.
            # The last stage's products are deferred one more stage so the
            # in-order DVE never waits on the attn-gather DMA latency.
            if tt < TT - 1:
                emit_ctx_prods(b, tt)
            else:
                late_ctx.append(b)
                att_f = out_pool.tile([1, T], f32, tag="attf")
                nc.vector.tensor_scalar_mul(att_f[:], s["e_f"][:], s["rs"][:])
                nc.sync.dma_start(attn_d[b:b + 1, :], att_f[:])

        def emit_ctx_prods(b, tt):
            s = st[b]
            for t in range(tt * TCS, (tt + 1) * TCS):
                prod = prod_pool.tile([128, H], f16, tag="prod")
                nc.vector.tensor_scalar_mul(prod[:], s["val"][:, t, :],
                                            s["aT"][:, t:t + 1])
                ctx_chunk(b, t, prod)

        def flush_late_ctx():
            while late_ctx:
                emit_ctx_prods(late_ctx.pop(0), TT - 1)

        def ctx_pe(b):
            """partition-reduce + normalize + store context (PE/ACT tail)."""
            s = st.pop(b)
            cs = out_pool.tile([1, H], f32, tag="cs")
            for vh in range(2):
                cp = vp_psum.tile([1, NT], f32, tag="vp")
                nc.tensor.matmul(cp[:], ones16[:],
                                 s["cav16"][:, vh * NT:(vh + 1) * NT],
                                 start=True, stop=True)
                # fold the softmax 1/sum into the copy out of PSUM
                nc.scalar.activation(cs[:, vh * NT:(vh + 1) * NT], cp[:],
                                     Copy, scale=s["rs"][:])
            nc.sync.dma_start(ctx_d[b:b + 1, :], cs[:])

        from collections import deque
        pending = deque()   # (b, tt) blocks whose reduce stage is deferred
        late_ctx = []       # batches whose tt3 ctx products are deferred

        for b in range(NB):
            e_f = ef_pool.tile([1, T], f32, tag="ef", name=f"ef{b}")
            S4 = sm_pool.tile([1, TT], f32, tag="S4", name=f"S4_{b}")
            aT = at_pool.tile([128, TC], f32, tag="aT", name=f"aT{b}")
            caccv = accv_pool.tile([128, H], f16, tag="accv", name=f"cv{b}")
            cav16 = av16_pool.tile([128, H], f16, tag="av16", name=f"cv16_{b}")
            st[b] = {"kt": dict(kt_first) if b == 0 else {}, "a16": {},
                     "e_f": e_f, "S4": S4, "aT": aT,
                     "caccv": caccv, "cav16": cav16}

            for tt in range(TT):
                if not (b == 0 and tt == 0):
                    for ko in range(KO):
                        kc = kt_pool.tile([128, NT], bf16, tag="kt",
                                          name=f"kt{b}_{ko}_{tt}")
                        nc.sync.dma_start(kc[:],
                                          kT3[b, ko][:, tt * NT:(tt + 1) * NT])
                        st[b]["kt"][(ko, tt)] = kc
                tt_block(b, tt)
                if tt == 0:
                    val_t = val_pool.tile([128, TC, H], bf16, tag="val",
                                          name=f"val{b}")
                    st[b]["val"] = val_t
                # values arrive in 1MB chunks, each two stages before its use
                nc.sync.dma_start(
                    st[b]["val"][:, tt * TCS:(tt + 1) * TCS, :],
                    val3[b][:, tt * TCS:(tt + 1) * TCS, :])
                pending.append((b, tt))
                if len(pending) > 2:
                    reduce_stage(*pending.popleft())
                    if pending[-1][1] == 2:
                        # one stage after the previous batch's tt3 reduce:
                        # its attn gather has long landed
                        flush_late_ctx()
                # drain one stage early at the very end so the epilogue only
                # has a single serialized reduce chain left
                if b == NB - 1 and tt == TT - 1:
                    reduce_stage(*pending.popleft())
                # free this t-tile's key chunks
                for ko in range(KO):
                    del st[b]["kt"][(ko, tt)]
            if b > 0:
                ctx_pe(b - 1)

        while pending:
            reduce_stage(*pending.popleft())
        flush_late_ctx()
        ctx_pe(NB - 1)

    nc.compile()
    return nc


def _prep(query, keys, values, Wq, bq, Wk, bk, Wv, bv):
    qb = (query.astype(np.float32) @ Wq.astype(np.float32)
          + bq.astype(np.float32) + bk.astype(np.float32))       # [B, H]
    wk_bf = np.ascontiguousarray(Wk.astype(BF16))
    wvT = np.ascontiguousarray(Wv.reshape(HT, 128).T.astype(np.float32))
    in_maps = []
    for c in range(N_CORES):
        sl = slice(c * NB, (c + 1) * NB)
        kT = np.ascontiguousarray(
            keys[sl].transpose(0, 2, 1).astype(BF16))            # [NB, K, T]
        vals = np.ascontiguousarray(values[sl].astype(BF16))     # [NB, T, H]
        qbT = np.ascontiguousarray(
            qb[sl].reshape(NB, HT, 128).transpose(2, 0, 1).reshape(128, NB * HT))
        in_maps.append({"kT": kT, "vals": vals, "wk": wk_bf,
                        "qbT": qbT, "wvT": wvT})
    return in_maps


def kernel(query, keys, values, Wq, bq, Wk, bk, Wv, bv):
    from concourse.bass_utils import run_bass_kernel_spmd

    if "nc" not in _CACHE:
        _CACHE["nc"] = _build()
    nc = _CACHE["nc"]

    in_maps = _prep(query, keys, values, Wq, bq, Wk, bk, Wv, bv)
    trace = bool(int(os.environ.get("KERNEL_TRACE", "0")))
    res = run_bass_kernel_spmd(nc, in_maps, core_ids=list(range(N_CORES)),
                               trace=trace)
    _CACHE["last_exec_ns"] = res.exec_time_ns
    _CACHE["last_results"] = res

    context = np.concatenate([res.results[c]["ctx"] for c in range(N_CORES)], axis=0)
    attn = np.concatenate([res.results[c]["attn"] for c in range(N_CORES)], axis=0)
    return context.astype(np.float32), attn.astype(np.float32)


# revision 26
# speedup vs baseline: 1.2422x; 1.0147x over previous
"""Additive (Bahdanau) attention kernel for 8 Trainium2 NeuronCores.

Problem shapes (hardcoded): B=64, T=2048, Q_DIM=K_DIM=H_DIM=1024.
  q_proj = query @ Wq + bq                      (B, H)
  k_proj = keys @ Wk + bk                       (B, T, H)
  energy = tanh(q_proj[:, None, :] + k_proj)    (B, T, H)
  scores = energy @ Wv + bv                     (B, T)
  attn   = softmax(scores, axis=1)              (B, T)
  context= attn @ values                        (B, V)
returns (context, attn).

Strategy: data-parallel over batch, 8 batches per core.
 - Host: fold q_proj+bq+bk into a per-(batch,h) bias (fp32, exact); bv
   drops out of the softmax entirely. Cast keys/values/Wk to bf16 and
   pre-transpose keys to [K, T] per batch so the device only does clean
   contiguous DMAs (128KB chunks for fine-grained prefetch).
 - Device per batch: k_proj via PE matmuls with Wk stationary
   ([h partitions, t free] orientation, fp32 PSUM accum over K chunks);
   tanh+bias fused in one ScalarE activation per tile (bias is
   per-partition = per-h).
 - Scores: ScalarE scales energy by Wv (per-partition scale), DVE sums
   the 8 h-chunks (fp32 chain, last add emits fp16), and a single
   ones-vector fp16 matmul does the 128-partition reduction at full
   streaming rate. The reduce work is emitted TWO t-tile stages late so
   the in-order PE never waits on the ACT/DVE chain.
 - Softmax online, no max subtraction (|scores| <= sum|Wv| < 32, exp is
   safe in fp32): exp per t-tile straight out of PSUM with accum_out
   partial sums. The 1/sum normalization folds into the final scale of
   attn and context.
 - Context: DVE multiplies values (native [t, v] layout) by the
   unnormalized attn weight (per-partition scalar, via a tiny DRAM
   transpose roundtrip), 4 t-chunks per reduce stage, fp32 chain with a
   fp16 tail, ones-fp16 matmul reduces partitions one batch later.
"""

import os
import sys
from contextlib import ExitStack

for _p in ("/opt/trn_rl_repo", "/root/.axon_site/_ro/trn_rl_repo"):
    if os.path.isdir(_p) and _p not in sys.path:
        sys.path.append(_p)

import numpy as np
import ml_dtypes

BF16 = ml_dtypes.bfloat16

N_CORES = 8
B, T, H, KD = 64, 2048, 1024, 1024
NB = B // N_CORES          # batches per core
KO = KD // 128             # contraction chunks (k on partitions)
HT = H // 128              # h tiles (output partitions)
NT = 512                   # matmul moving free dim
TT = T // NT               # t tiles per batch
TC = T // 128              # t chunks (contraction for context)
TCS = TC // TT             # t chunks per t tile (4)

_CACHE = {}


def _build():
    """Build + bacc-compile the SPMD Bass program (once per process)."""
    import concourse.bacc as bacc
    import concourse.tile as tile
    from concourse import mybir

    f32 = mybir.dt.float32
    f16 = mybir.dt.float16
    bf16 = mybir.dt.bfloat16
    Tanh = mybir.ActivationFunctionType.Tanh
    Exp = mybir.ActivationFunctionType.Exp
    Copy = mybir.ActivationFunctionType.Copy
    add = mybir.AluOpType.add

    nc = bacc.Bacc("TRN2", target_bir_lowering=False, debug=False,
                   num_devices=N_CORES)

    kT_d = nc.dram_tensor("kT", [NB, KD, T], bf16, kind="ExternalInput").ap()
    val_d = nc.dram_tensor("vals", [NB, T, H], bf16, kind="ExternalInput").ap()
    wk_d = nc.dram_tensor("wk", [KD, H], bf16, kind="ExternalInput").ap()
    qbT_d = nc.dram_tensor("qbT", [128, NB * HT], f32, kind="ExternalInput").ap()
    wvT_d = nc.dram_tensor("wvT", [128, HT], f32, kind="ExternalInput").ap()
    ctx_d = nc.dram_tensor("ctx", [NB, H], f32, kind="ExternalOutput").ap()
    attn_d = nc.dram_tensor("attn", [NB, T], f32, kind="ExternalOutput").ap()
    ef_d = nc.dram_tensor("ef", [NB, T], f32).ap()      # internal scratch

    kT3 = kT_d.rearrange("b (ko p) t -> b ko p t", p=128)
    val3 = val_d.rearrange("b (tc p) v -> b p tc v", p=128)
    wk3 = wk_d.rearrange("(ko p) h -> ko p h", p=128)

    with tile.TileContext(nc) as tc, ExitStack() as ctx:
        const = ctx.enter_context(tc.tile_pool(name="const", bufs=1))
        kt_pool = ctx.enter_context(tc.tile_pool(name="kt", bufs=20))
        val_pool = ctx.enter_context(tc.tile_pool(name="val", bufs=2))
        en_pool = ctx.enter_context(tc.tile_pool(name="en", bufs=6))
        enw_pool = ctx.enter_context(tc.tile_pool(name="enw", bufs=8))
        acc_pool = ctx.enter_context(tc.tile_pool(name="acc", bufs=3))
        a16_pool = ctx.enter_context(tc.tile_pool(name="a16", bufs=3))
        prod_pool = ctx.enter_context(tc.tile_pool(name="prod", bufs=5))
        accv_pool = ctx.enter_context(tc.tile_pool(name="accv", bufs=2))
        av16_pool = ctx.enter_context(tc.tile_pool(name="av16", bufs=2))
        at_pool = ctx.enter_context(tc.tile_pool(name="at", bufs=3))
        sm_pool = ctx.enter_context(tc.tile_pool(name="sm", bufs=3))
        ef_pool = ctx.enter_context(tc.tile_pool(name="efp", bufs=2))
        out_pool = ctx.enter_context(tc.tile_pool(name="outp", bufs=2))
        kp_psum = ctx.enter_context(tc.tile_pool(name="kp", bufs=5, space="PSUM"))
        vp_psum = ctx.enter_context(tc.tile_pool(name="vp", bufs=2, space="PSUM"))
        tp_psum = ctx.enter_context(tc.tile_pool(name="tp", bufs=1, space="PSUM"))

        # prefetch the very first k-chunk tiles ahead of the fat constant
        # DMAs so the PE can start within a few microseconds
        kt_first = {}
        for ko in range(KO):
            kc = kt_pool.tile([128, NT], bf16, tag="kt", name=f"kt0_{ko}_0")
            nc.sync.dma_start(kc[:], kT3[0, ko][:, 0:NT])
            kt_first[(ko, 0)] = kc
        # resident constants; Wk in per-ko chunks so the first matmul only
        # waits on a 256KB DMA
        wk_sb = []
        for ko in range(KO):
            w = const.tile([128, H], bf16, tag=f"wk{ko}")
            nc.sync.dma_start(w[:], wk3[ko])
            wk_sb.append(w)
        qbT_sb = const.tile([128, NB * HT], f32, tag="qbT")
        nc.sync.dma_start(qbT_sb[:], qbT_d[:])
        wvT_sb = const.tile([128, HT], f32, tag="wvT")
        nc.sync.dma_start(wvT_sb[:], wvT_d[:])
        ones16 = const.tile([128, 1], f16, tag="ones16")
        nc.vector.memset(ones16[:], 1.0)
        one11 = const.tile([1, 1], f32, tag="one11")
        nc.vector.memset(one11[:], 1.0)

        # per-batch state
        st = {}   # b -> dict(kt{}, a16{tt}, e_f, S4, aT, rs, val, ...)

        def tt_block(b, tt):
            """k_proj + tanh + Wv-scale + h-chunk accumulation for (b, tt)."""
            s = st[b]
            tsl = slice(tt * NT, (tt + 1) * NT)
            acc = acc_pool.tile([128, NT], f16, tag="acc", name=f"acc{b}_{tt}")
            a16 = a16_pool.tile([128, NT], f16, tag="a16", name=f"a16_{b}_{tt}")
            s["a16"][tt] = a16
            enw_prev = None
            for ht in range(HT):
                kp = kp_psum.tile([128, NT], f32, tag="kp")
                for ko in range(KO):
                    nc.tensor.matmul(
                        kp[:],
                        wk_sb[ko][:, ht * 128:(ht + 1) * 128],
                        s["kt"][(ko, tt)][:],
                        start=(ko == 0), stop=(ko == KO - 1))
                en = en_pool.tile([128, NT], bf16, tag="en")
                nc.scalar.activation(
                    en[:], kp[:], Tanh,
                    bias=qbT_sb[:, b * HT + ht:b * HT + ht + 1])
                enw = enw_pool.tile([128, NT], f16, tag="enw")
                nc.vector.tensor_scalar_mul(enw[:], en[:],
                                            wvT_sb[:, ht:ht + 1])
                if ht == 0:
                    enw_prev = enw
                elif ht == 1:
                    nc.vector.tensor_tensor(acc[:], enw_prev[:], enw[:], add)
                elif ht < HT - 1:
                    nc.vector.tensor_tensor(acc[:], acc[:], enw[:], add)
                else:
                    nc.vector.tensor_tensor(a16[:], acc[:], enw[:], add)

        def ctx_chunk(b, t, prod):
            """advance the context accumulation chain by one t-chunk."""
            s = st[b]
            if t == 0:
                s["cprev"] = prod
            elif t == 1:
                nc.vector.tensor_tensor(s["caccv"][:], s["cprev"][:],
                                        prod[:], add)
            elif t < TC - 1:
                nc.vector.tensor_tensor(s["caccv"][:], s["caccv"][:],
                                        prod[:], add)
            else:
                nc.vector.tensor_tensor(s["cav16"][:], s["caccv"][:],
                                        prod[:], add)

        def reduce_stage(b, tt):
            """Partition-reduce + exp + attn scatter for (b, tt) + context
            chunk muls; emitted two t-tile stages late so the in-order PE
            never waits on the ACT/DVE chain."""
            s = st[b]
            tsl = slice(tt * NT, (tt + 1) * NT)
            a16 = s["a16"].pop(tt)
            spt = vp_psum.tile([1, NT], f32, tag="vp")
            nc.tensor.matmul(spt[:], ones16[:], a16[:], start=True, stop=True)
            nc.scalar.activation(s["e_f"][:, tsl], spt[:], Exp,
                                 accum_out=s["S4"][:, tt:tt + 1])
            if b == NB - 1:
                # tail path: PE-transpose the 4 attn chunks (no DMA latency)
                atp = tp_psum.tile([128, TCS], f32, tag="tp")
                for j in range(TCS):
                    tch = (tt * TCS + j) * 128
                    nc.tensor.matmul(atp[:, j:j + 1],
                                     s["e_f"][0:1, tch:tch + 128], one11[:],
                                     is_transpose=True, start=True, stop=True)
                nc.scalar.copy(s["aT"][:, tt * TCS:(tt + 1) * TCS], atp[:])
            else:
                nc.sync.dma_start(ef_d[b:b + 1, tsl], s["e_f"][:, tsl])
                with nc.allow_non_contiguous_dma(reason="tiny 2KB attn transpose"):
                    nc.gpsimd.dma_start(
                        s["aT"][:, tt * TCS:(tt + 1) * TCS],
                        ef_d[b, tsl].rearrange("(c p) -> p c", p=128))
            if tt == TT - 1:
                # batch fully scored: finalize the softmax scalars first so
                # the context chain below never waits on the attn output
                S = sm_pool.tile([1, 1], f32, tag="S")
                nc.vector.reduce_sum(S[:], s["S4"][:], axis=mybir.AxisListType.X)
                rs = sm_pool.tile([1, 1], f32, tag="rs")
                nc.vector.reciprocal(rs[:], S[:])
                s["rs"] = rs
            # context products for the 4 t-chunks just scattered into aT.
            # The last stage's products are deferred one more stage so the
            # in-order DVE never waits on the attn-gather DMA latency.
            if tt < TT - 1:
                emit_ctx_prods(b, tt)
            else:
                late_ctx.append(b)
                att_f = out_pool.tile([1, T], f32, tag="attf")
                nc.vector.tensor_scalar_mul(att_f[:], s["e_f"][:], s["rs"][:])
                nc.sync.dma_start(attn_d[b:b + 1, :], att_f[:])

        def emit_ctx_prods(b, tt):
            s = st[b]
            for t in range(tt * TCS, (tt + 1) * TCS):
                prod = prod_pool.tile([128, H], f16, tag="prod")
                nc.vector.tensor_scalar_mul(prod[:], s["val"][:, t, :],
                                            s["aT"][:, t:t + 1])
                ctx_chunk(b, t, prod)

        def flush_late_ctx():
            while late_ctx:
                emit_ctx_prods(late_ctx.pop(0), TT - 1)

        def ctx_pe(b):
            """partition-reduce + normalize + store context (PE/ACT tail)."""
            s = st.pop(b)
            cs = out_pool.tile([1, H], f32, tag="cs")
            for vh in range(2):
                cp = vp_psum.tile([1, NT], f32, tag="vp")
                nc.tensor.matmul(cp[:], ones16[:],
                                 s["cav16"][:, vh * NT:(vh + 1) * NT],
                                 start=True, stop=True)
                # fold the softmax 1/sum into the copy out of PSUM
                nc.scalar.activation(cs[:, vh * NT:(vh + 1) * NT], cp[:],
                                     Copy, scale=s["rs"][:])
            nc.sync.dma_start(ctx_d[b:b + 1, :], cs[:])

        from collections import deque
        pending = deque()   # (b, tt) blocks whose reduce stage is deferred
        late_ctx = []       # batches whose tt3 ctx products are deferred

        for b in range(NB):
            e_f = ef_pool.tile([1, T], f32, tag="ef", name=f"ef{b}")
            S4 = sm_pool.tile([1, TT], f32, tag="S4", name=f"S4_{b}")
            aT = at_pool.tile([128, TC], f32, tag="aT", name=f"aT{b}")
            caccv = accv_pool.tile([128, H], f16, tag="accv", name=f"cv{b}")
            cav16 = av16_pool.tile([128, H], f16, tag="av16", name=f"cv16_{b}")
            st[b] = {"kt": dict(kt_first) if b == 0 else {}, "a16": {},
                     "e_f": e_f, "S4": S4, "aT": aT,
                     "caccv": caccv, "cav16": cav16}

            for tt in range(TT):
                if not (b == 0 and tt == 0):
                    for ko in range(KO):
                        kc = kt_pool.tile([128, NT], bf16, tag="kt",
                                          name=f"kt{b}_{ko}_{tt}")
                        nc.sync.dma_start(kc[:],
                                          kT3[b, ko][:, tt * NT:(tt + 1) * NT])
                        st[b]["kt"][(ko, tt)] = kc
                tt_block(b, tt)
                if tt == 0:
                    val_t = val_pool.tile([128, TC, H], bf16, tag="val",
                                          name=f"val{b}")
                    st[b]["val"] = val_t
                # values arrive in 1MB chunks, each two stages before its use
                nc.sync.dma_start(
                    st[b]["val"][:, tt * TCS:(tt + 1) * TCS, :],
                    val3[b][:, tt * TCS:(tt + 1) * TCS, :])
                pending.append((b, tt))
                if len(pending) > 2:
                    reduce_stage(*pending.popleft())
                    if pending[-1][1] == 2:
                        # one stage after the previous batch's tt3 reduce:
                        # its attn gather has long landed
                        flush_late_ctx()
                # drain one stage early at the very end so the epilogue only
                # has a single serialized reduce chain left
                if b == NB - 1 and tt == TT - 1:
                    reduce_stage(*pending.popleft())
                # free this t-tile's key chunks
                for ko in range(KO):
                    del st[b]["kt"][(ko, tt)]
            if b > 0:
                ctx_pe(b - 1)

        while pending:
            reduce_stage(*pending.popleft())
        flush_late_ctx()
        ctx_pe(NB - 1)

    nc.compile()
    return nc


def _prep(query, keys, values, Wq, bq, Wk, bk, Wv, bv):
    qb = (query.astype(np.float32) @ Wq.astype(np.float32)
          + bq.astype(np.float32) + bk.astype(np.float32))       # [B, H]
    wk_bf = np.ascontiguousarray(Wk.astype(BF16))
    wvT = np.ascontiguousarray(Wv.reshape(HT, 128).T.astype(np.float32))
    in_maps = []
    for c in range(N_CORES):
        sl = slice(c * NB, (c + 1) * NB)
        kT = np.ascontiguousarray(
            keys[sl].transpose(0, 2, 1).astype(BF16))            # [NB, K, T]
        vals = np.ascontiguousarray(values[sl].astype(BF16))     # [NB, T, H]
        qbT = np.ascontiguousarray(
            qb[sl].reshape(NB, HT, 128).transpose(2, 0, 1).reshape(128, NB * HT))
        in_maps.append({"kT": kT, "vals": vals, "wk": wk_bf,
                        "qbT": qbT, "wvT": wvT})
    return in_maps


def kernel(query, keys, values, Wq, bq, Wk, bk, Wv, bv):
    from concourse.bass_utils import run_bass_kernel_spmd

    if "nc" not in _CACHE:
        _CACHE["nc"] = _build()
    nc = _CACHE["nc"]

    in_maps = _prep(query, keys, values, Wq, bq, Wk, bk, Wv, bv)
    trace = bool(int(os.environ.get("KERNEL_TRACE", "0")))
    res = run_bass_kernel_spmd(nc, in_maps, core_ids=list(range(N_CORES)),
                               trace=trace)
    _CACHE["last_exec_ns"] = res.exec_time_ns
    _CACHE["last_results"] = res

    context = np.concatenate([res.results[c]["ctx"] for c in range(N_CORES)], axis=0)
    attn = np.concatenate([res.results[c]["attn"] for c in range(N_CORES)], axis=0)
    return context.astype(np.float32), attn.astype(np.float32)
